# revision 69
# baseline (speedup 1.0000x reference)
"""Trainium2 Bass kernel for nn_MHAAttention (LayerNorm2d + MHA w/ rel-pos bias + residual).

Sharding: data-parallel over batch - 8 batch elements, one per NeuronCore.
No collectives needed. ~3x faster than the fp32 baseline (632us -> ~211us).

Design notes (all matmuls bf16: 1 cycle/row on the PE vs fp32's 4):
  - LN folded into the projection weights (exact algebra); stats via
    ones-matmuls on a bf16 cast of x; the per-token 1/sqrt(var) and -mu*rs
    are computed on a DMA-redistributed (128, 8) layout because the DVE
    reciprocal costs ~6 cycles per FREE element, then replicated across
    partitions with K=1 ones-matmuls.
  - scores computed per head-pair via PE row-tiling: head 2g's K=64
    contraction on array rows 0-63, head 2g+1 on rows 64-127 — the two
    matmul streams partially overlap on the 128x128 array.
  - rel-pos bias applied multiplicatively AFTER exp: aT = exp(s) * expstrip
    (expstrip = exp(bias strip), host-precomputed bf16, block-Toeplitz
    compressed to a (128, 1920) strip per head) so the DVE op runs in the
    2x bf16 mode instead of a 1x fp32 add against PSUM.
  - softmax denominator Z from a ones-augmented V column; 1/Z also via the
    DMA-redistribute-small trick; normalization pipelined one pair late so
    the DMA round trip hides behind PE work.
  - attn@V for the even head streams behind the exps; for the odd head it
    runs 2 jt-iterations behind on stored aT tiles, so the PE queue never
    waits on the vector engines.
  - Q/K projections of the NEXT pair and dummy warm-up matmuls are
    interleaved as PE gap-filler: the HAM clock gate re-throttles the PE
    array to 1.2 GHz after any idle window, so the stream must stay dense.
  - output projection with K=128 head-pair stacking; odd-head oT shifted to
    partitions 64-127 via SBUF-to-SBUF DMA.
"""

import sys

for _p in ("/opt/trn_rl_repo",):
    if _p not in sys.path:
        sys.path.insert(0, _p)

from contextlib import ExitStack

import numpy as np
import ml_dtypes

import concourse.bass as bass
import concourse.mybir as mybir
import concourse.tile as tile
from concourse.bass_utils import run_bass_kernel_spmd

F32 = mybir.dt.float32
F32R = mybir.dt.float32r
BF16 = mybir.dt.bfloat16
AF = mybir.ActivationFunctionType
OP = mybir.AluOpType

B = 8
CH = 512
H = W = 32
NT = H * W          # 1024 tokens
HEADS = 8
HD = 64
EPS = 1e-6
P = 128
CT = CH // P        # 4 channel tiles
TT = NT // P        # 8 token tiles
IC = NT // 512      # 2 free-dim chunks of 512
NP = HEADS // 2     # 4 head pairs
STRIP_W = 60 * 32   # 1920


def _build_strips(rel: np.ndarray) -> np.ndarray:
    """(3969, 8) rel table -> (8, 128, 1920) bias strips.

    strip[h, 32*jh_l + jw, 32*g + iw] = T_h[g - jh_l + 3, iw - jw + 31]
    where T_h = rel[:, h].reshape(63, 63).
    bias.T block for key-tile jt is then strip[:, (28-4*jt)*32 : +1024].
    """
    T = rel.reshape(63, 63, HEADS)  # [a, b, h]
    jh_l = np.arange(4)[:, None, None, None]
    jw = np.arange(32)[None, :, None, None]
    g = np.arange(60)[None, None, :, None]
    iw = np.arange(32)[None, None, None, :]
    a = g - jh_l + 3          # in [0,62]
    b = iw - jw + 31          # in [0,62]
    a_b, b_b = np.broadcast_arrays(a, b)
    out = T[a_b, b_b, :]      # (4, 32, 60, 32, 8)
    out = np.ascontiguousarray(np.moveaxis(out, -1, 0)).reshape(HEADS, P, STRIP_W)
    return out.astype(np.float32)


def _build_nc() -> bass.Bass:
    nc = bass.Bass()

    x_d = nc.declare_dram_parameter("x", [CH, NT], F32, isOutput=False)
    wqT_d = nc.declare_dram_parameter("wqT", [CH, CH], BF16, isOutput=False)
    wkT_d = nc.declare_dram_parameter("wkT", [CH, CH], BF16, isOutput=False)
    wvT_d = nc.declare_dram_parameter("wvT", [CH, CH], BF16, isOutput=False)
    wpP_d = nc.declare_dram_parameter("wpP", [P, NP, CH], BF16, isOutput=False)
    bqk_d = nc.declare_dram_parameter("bqk", [P, 2, CT], F32, isOutput=False)
    bp_d = nc.declare_dram_parameter("bp", [P, CT], F32, isOutput=False)
    bv_d = nc.declare_dram_parameter("bv", [1, CH], BF16, isOutput=False)
    estr_d = nc.declare_dram_parameter("estr", [HEADS, P, STRIP_W], BF16, isOutput=False)
    y_d = nc.declare_dram_parameter("y", [CH, NT], F32, isOutput=True)

    with tile.TileContext(nc) as tc, ExitStack() as ctx:
        singles = ctx.enter_context(tc.tile_pool(name="singles", bufs=1))
        work = ctx.enter_context(tc.tile_pool(name="work", bufs=2))
        strip_pool = ctx.enter_context(tc.tile_pool(name="strip_pool", bufs=4))
        a_pool = ctx.enter_context(tc.tile_pool(name="a_pool", bufs=4))
        ah1_pool = ctx.enter_context(tc.tile_pool(name="ah1_pool", bufs=6))
        # PSUM: psM 2 slots x (128,1024)f32 = 4 banks; psO 2 x (65,1024) = 4.
        psM = ctx.enter_context(tc.tile_pool(name="psM", bufs=2, space="PSUM"))
        psO = ctx.enter_context(tc.tile_pool(name="psO", bufs=2, space="PSUM"))

        # ---------- persistent SBUF ----------
        x_sb = singles.tile([P, CT, NT], F32)        # raw x (residual + LN apply)
        xb_sb = singles.tile([P, CT, NT], BF16)      # LN output (normalized bf16)
        qT_sb = singles.tile([P, CT, NT], BF16)      # (d part, t free), pair-stacked
        kT_sb = singles.tile([P, CT, NT], BF16)
        v_sb = singles.tile([P, TT, HEADS * (HD + 1)], BF16)  # per head [v(64)|1]
        oTP_sb = singles.tile([P, NP, NT], BF16)     # pair-stacked normalized oT
        wpP_sb = singles.tile([P, NP, CH], BF16)
        bqk_sb = singles.tile([P, 2, CT], F32)
        bp_sb = singles.tile([P, CT], F32)
        bv_sb = singles.tile([1, CH], BF16)
        bvb_sb = singles.tile([P, CH], BF16)         # bv broadcast across partitions
        ones_mat = singles.tile([P, P], BF16)
        ones_row = singles.tile([1, P], BF16)
        eps_sb = singles.tile([P, 1], F32)
        nc.vector.memset(eps_sb[:], float(EPS))
        lnsm_sb = singles.tile([P, 2, 8], F32)       # LN mu/ve small layout
        lnsmb_sb = singles.tile([P, 2, 8], BF16)     # LN rs,b small bf16
        lnrow_sb = singles.tile([1, 2, NT], BF16)    # rs,b rows at partition 0
        lnmv_sb = singles.tile([1, 2, NT], F32)      # mu,ve rows at partition 0
        zrowZ_sb = singles.tile([HD + 1, 2, NT], F32)  # Z rows evac'd at part. 64
        zsm_sb = singles.tile([P, 2, 8], F32)        # Z small layout (per pair)
        zrb_sb = singles.tile([P, 2, 8], BF16)       # 1/Z small, bf16
        zrow_sb = singles.tile([1, 2, NT], BF16)     # 1/Z rows at partition 0

        # x first, in per-ct chunks — the LN cast chases the chunks
        x_r = x_d.rearrange("(ct p) t -> ct p t", p=P)
        for ct in range(CT):
            nc.sync.dma_start(x_sb[:, ct], x_r[ct])
        nc.vector.memset(ones_mat[:], 1.0)
        nc.vector.memset(ones_row[:], 1.0)
        nc.sync.dma_start(bqk_sb[:], bqk_d[:])
        nc.sync.dma_start(bp_sb[:], bp_d[:])
        nc.sync.dma_start(bv_sb[:], bv_d[:])
        nc.sync.dma_start(wpP_sb[:], wpP_d[:])

        # ones columns of v_aug
        v_view = v_sb[:].rearrange("p tt (h w) -> p tt h w", w=HD + 1)
        nc.vector.memset(v_view[:, :, :, HD : HD + 1], 1.0)

        wqT_sb = singles.tile([P, CT, CH], BF16)
        wkT_sb = singles.tile([P, CT, CH], BF16)
        wvT_sb = singles.tile([P, CT, CH], BF16)
        nc.sync.dma_start(wqT_sb[:], wqT_d.rearrange("(ck p) d -> p ck d", p=P))
        nc.sync.dma_start(wkT_sb[:], wkT_d.rearrange("(ck p) d -> p ck d", p=P))
        nc.sync.dma_start(wvT_sb[:], wvT_d.rearrange("(ck p) d -> p ck d", p=P))

        # strips for pairs 0,1 DMA'd in the prologue; pairs 2,3 prefetched
        # from inside the pair loop (keeps the sync queue from stalling on
        # the strip-slot WAR semaphore ahead of the z-chain DMAs)
        estr_tiles = []
        for h in range(HEADS):
            st = strip_pool.tile([P, STRIP_W], BF16, name=f"estr_{h}", tag="strip")
            if h < 4:
                nc.sync.dma_start(st[:], estr_d[h])
            estr_tiles.append(st)

        # PE warmup: dummy matmuls during the x DMA (HAM un-throttle needs
        # ~3.4us of sustained PE activity; these overlap the input DMA).
        warm_ps = psM.tile([P, NT], F32, tag="s")
        for i in range(16):
            nc.tensor.matmul(warm_ps[:, :P], lhsT=ones_mat[:], rhs=ones_mat[:],
                             start=True, stop=True, skip_group_check=True)
        # bv broadcast across partitions (K=1 ones-column matmul + ACT evac)
        nc.tensor.matmul(warm_ps[:, :CH], lhsT=ones_row[:], rhs=bv_sb[:],
                         start=True, stop=True, skip_group_check=True)
        nc.scalar.activation(out=bvb_sb[:], in_=warm_ps[:, :CH], func=AF.Copy)

        # ---------- phase 1: LayerNorm ----------
        # bf16 cast of raw x (DVE, pipelined with the x DMA) + squares on
        # ACT (idle in this phase); stats via bf16 ones-matmuls
        with tc.tile_pool(name="ln_pool", bufs=2) as lnp, \
             tc.tile_pool(name="ln_single", bufs=1) as lns:
            # xb_sb temporarily holds the raw-x bf16 cast (overwritten by the
            # normalized output after the stats matmuls complete); cast,
            # square, and stats matmuls interleaved per channel tile
            sum_ps = psM.tile([P, NT], F32, tag="s")
            sq_ps = psM.tile([P, NT], F32, tag="s")
            for ct in range(CT):
                nc.vector.tensor_copy(out=xb_sb[:, ct], in_=x_sb[:, ct])
                x2 = lnp.tile([P, NT], BF16, name=f"x2_{ct}", tag="x2", bufs=2)
                nc.scalar.activation(out=x2[:], in_=xb_sb[:, ct], func=AF.Square)
                for ic in range(IC):
                    sl = slice(ic * 512, ic * 512 + 512)
                    nc.tensor.matmul(sum_ps[:, sl], lhsT=ones_mat[:],
                                     rhs=xb_sb[:, ct, sl],
                                     start=(ct == 0), stop=(ct == CT - 1))
                    nc.tensor.matmul(sq_ps[:, sl], lhsT=ones_mat[:],
                                     rhs=x2[:, sl],
                                     start=(ct == 0), stop=(ct == CT - 1))

            # LN scalars on a DMA-redistributed (128, 8) small layout: the
            # replicated rows of sum/sq go through ACT copies, one row is
            # DMA'd small, rs = 1/sqrt(var+eps) and b = -mu*rs cost ~100ns
            # each there (DVE recip is ~6 cyc per FREE elem), then rows are
            # DMA'd back and partition-broadcast by a stride-0 DMA.
            rs_bc = lns.tile([P, NT], F32)
            b_bc = lns.tile([P, NT], F32)
            nc.scalar.activation(out=lnmv_sb[:, 0, :], in_=sum_ps[0:1, :],
                                 func=AF.Copy, scale=1.0 / CH)
            nc.scalar.activation(out=lnmv_sb[:, 1, :], in_=sq_ps[0:1, :],
                                 func=AF.Copy, scale=1.0 / CH)
            for e in range(2):
                nc.scalar.dma_start(
                    lnsm_sb[:, e, :],
                    lnmv_sb[:, e, :].rearrange("o (p j) -> o p j", j=8))
            musq = lns.tile([P, 8], F32)
            var_s = lns.tile([P, 8], F32)
            rs_s = lns.tile([P, 8], F32)
            b_s = lns.tile([P, 8], F32)
            nc.vector.tensor_tensor(out=musq[:], in0=lnsm_sb[:, 0, :],
                                    in1=lnsm_sb[:, 0, :], op=OP.mult)
            nc.vector.tensor_tensor(out=var_s[:], in0=lnsm_sb[:, 1, :],
                                    in1=musq[:], op=OP.subtract)
            nc.scalar.activation(out=var_s[:], in_=var_s[:], func=AF.Sqrt,
                                 bias=eps_sb[:])
            nc.vector.reciprocal(out=rs_s[:], in_=var_s[:])
            nc.vector.scalar_tensor_tensor(out=b_s[:], in0=lnsm_sb[:, 0, :],
                                           scalar=-1.0, in1=rs_s[:],
                                           op0=OP.mult, op1=OP.mult)
            nc.vector.tensor_copy(out=lnsmb_sb[:, 0, :], in_=rs_s[:])
            nc.vector.tensor_copy(out=lnsmb_sb[:, 1, :], in_=b_s[:])
            for e in range(2):
                nc.gpsimd.dma_start(
                    lnrow_sb[:, e, :].rearrange("o (p j) -> o p j", j=8),
                    lnsmb_sb[:, e, :])
            # HAM filler: keep the PE array active across the LN small-DMA
            # chain so the V/QK phases start at full clock
            warm2 = psM.tile([P, NT], F32, name="warm2", tag="s")
            for i in range(28):
                nc.tensor.matmul(warm2[:, :P], lhsT=ones_mat[:], rhs=ones_mat[:],
                                 start=True, stop=True, skip_group_check=True)
            # replicate rs,b across partitions: K=1 matmuls + ACT evacuation
            rep_ps = psM.tile([P, NT], F32, name="lnrep_ps", tag="s")
            bep_ps = psM.tile([P, NT], F32, name="lnbep_ps", tag="s")
            for ic in range(IC):
                sl = slice(ic * 512, ic * 512 + 512)
                nc.tensor.matmul(rep_ps[:, sl], lhsT=ones_mat[0:1, :],
                                 rhs=lnrow_sb[:, 0, sl], start=True, stop=True)
                nc.tensor.matmul(bep_ps[:, sl], lhsT=ones_mat[0:1, :],
                                 rhs=lnrow_sb[:, 1, sl], start=True, stop=True)
            nc.scalar.activation(out=rs_bc[:], in_=rep_ps[:], func=AF.Copy)
            nc.scalar.activation(out=b_bc[:], in_=bep_ps[:], func=AF.Copy)

            # apply xb = x*rs + b, split across DVE and GPSIMD (crosswise so
            # each ct's chain spans both engines and they run in parallel)
            for ct in range(CT):
                xm = lnp.tile([P, NT], F32, name=f"xm_{ct}", tag="xm", bufs=2)
                e_mul = nc.gpsimd if ct == 3 else nc.vector
                e_add = nc.gpsimd if ct == 2 else nc.vector
                e_mul.tensor_tensor(out=xm[:], in0=x_sb[:, ct], in1=rs_bc[:],
                                    op=OP.mult)
                e_add.tensor_tensor(out=xb_sb[:, ct], in0=xm[:], in1=b_bc[:],
                                    op=OP.add)

        # ---------- phase 2a: V projection ----------
        # token tiles 0,1 up front; 2..7 interleaved into pair 0's jt loop
        # (two iterations ahead of their first consumer) to absorb the
        # otherwise-serial V phase into the attention pipeline
        def emit_v(tt):
            tsl = slice(tt * P, tt * P + P)
            v_ps = psM.tile([P, NT], F32, name=f"v_ps_{tt}", tag="s")
            for ck in range(CT):
                nc.tensor.matmul(v_ps[:, :512], lhsT=xb_sb[:, ck, tsl],
                                 rhs=wvT_sb[:, ck, :],
                                 start=(ck == 0), stop=(ck == CT - 1))
            vp_v = v_ps[:, :512].rearrange("p (h w) -> p h w", w=HD)
            bv_v = bvb_sb[:].rearrange("p (h w) -> p h w", w=HD)
            nc.vector.scalar_tensor_tensor(
                out=v_view[:, tt, :, :HD], in0=vp_v, scalar=0.0,
                in1=bv_v, op0=OP.bypass, op1=OP.add)

        emit_v(0)
        emit_v(1)

        # ---------- phases 2b+3: per head pair: Q/K proj then attention ----------
        def emit_znorm(g, ob0, ob1):
            """Replicate 1/Z rows (K=1 matmuls) and normalize into oTP.

            Emitted one pair LATE (mid next pair's jt loop) so the z-chain's
            DMA round-trip latency is hidden behind PE work instead of
            stalling the in-order PE queue.
            """
            zr0 = psM.tile([P, NT], F32, name=f"zr0_{g}", tag="s")
            zr1 = psM.tile([P, NT], F32, name=f"zr1_{g}", tag="s")
            for ic in range(IC):
                sl = slice(ic * 512, ic * 512 + 512)
                nc.tensor.matmul(zr0[:HD, sl], lhsT=ones_mat[0:1, :HD],
                                 rhs=zrow_sb[:, 0, sl], start=True, stop=True)
                nc.tensor.matmul(zr1[:HD, sl], lhsT=ones_mat[0:1, :HD],
                                 rhs=zrow_sb[:, 1, sl], start=True, stop=True)
            nc.vector.tensor_tensor(out=oTP_sb[:HD, g], in0=ob0[:],
                                    in1=zr0[:HD, :], op=OP.mult)
            tmpO = work.tile([HD, NT], BF16, name=f"tmpO_{g}", tag="tmpO")
            nc.vector.tensor_tensor(out=tmpO[:], in0=ob1[:],
                                    in1=zr1[:HD, :], op=OP.mult)
            nc.gpsimd.dma_start(oTP_sb[HD:, g], tmpO[:])

        def emit_qk(gq, which, ic):
            """One Q or K projection chunk (4-MM accumulation + evac)."""
            sl = slice(ic * 512, ic * 512 + 512)
            dq = slice(gq * P, gq * P + P)
            w_sb = wqT_sb if which == 0 else wkT_sb
            dst = qT_sb if which == 0 else kT_sb
            ps = psM.tile([P, NT], F32, name=f"qk_ps_{gq}_{which}_{ic}", tag="s")
            for ck in range(CT):
                nc.tensor.matmul(ps[:, :512], lhsT=w_sb[:, ck, dq],
                                 rhs=xb_sb[:, ck, sl],
                                 start=(ck == 0), stop=(ck == CT - 1))
            nc.vector.tensor_scalar_add(out=dst[:, gq, sl], in0=ps[:, :512],
                                        scalar1=bqk_sb[:, which, gq : gq + 1])

        # Q/K for pair 0 up front
        for ic in range(IC):
            emit_qk(0, 0, ic)
            emit_qk(0, 1, ic)

        pend = None
        for g in range(NP):
            h0, h1 = 2 * g, 2 * g + 1
            # prefetch strips for pair g+2
            if g < 2:
                nc.sync.dma_start(estr_tiles[2 * g + 4][:], estr_d[2 * g + 4])
                nc.sync.dma_start(estr_tiles[2 * g + 5][:], estr_d[2 * g + 5])
            # Q/K chunks for pair g+1, interleaved into this pair's jt loop
            # (fills PE stall slivers so the HAM clock gate stays warm)
            qk_fill = ([(g + 1, w, ic) for ic in range(IC) for w in (0, 1)]
                       if g < NP - 1 else [])

            # attention for heads (h0: partitions 0-63, h1: 64-127)
            o0 = psO.tile([HD + 1, NT], F32, name=f"o0_{g}", tag="o")
            o1 = psO.tile([HD + 1, NT], F32, name=f"o1_{g}", tag="o")
            ah0 = {}
            ah1 = {}
            for jt in range(TT):
                jsl = slice(jt * P, jt * P + P)
                off = (28 - 4 * jt) * 32
                if g == 0 and jt < TT - 2:
                    emit_v(jt + 2)
                # pending z-normalization of the PREVIOUS pair (its DMA
                # chain has had ~3 jt iterations of PE work to complete)
                if jt == 3 and pend is not None:
                    emit_znorm(*pend)
                    pend = None
                # paired score matmuls: (0,0) and (64,0) row tiles run
                # concurrently on the PE array
                s0 = psM.tile([P, NT], F32, name=f"s0_{g}_{jt}", tag="s")
                s1 = psM.tile([P, NT], F32, name=f"s1_{g}_{jt}", tag="s")
                # grouped per head (A0,A1 then B0,B1): each head's lhsT loads
                # once and head B's LDWEIGHTS pulls ahead during A's MMs
                # (different row groups); a fully alternating order measured
                # ~35us WORSE (per-MM weight reloads, no pull-ahead)
                for ic in range(IC):
                    sl = slice(ic * 512, ic * 512 + 512)
                    nc.tensor.matmul(s0[:, sl], lhsT=kT_sb[:HD, g, jsl],
                                     rhs=qT_sb[:HD, g, sl], start=True, stop=True)
                for ic in range(IC):
                    sl = slice(ic * 512, ic * 512 + 512)
                    nc.tensor.matmul(s1[:, sl], lhsT=kT_sb[HD:, g, jsl],
                                     rhs=qT_sb[HD:, g, sl], start=True, stop=True)
                ah1[jt] = ah1_pool.tile([P, NT], BF16, name=f"ah1_{g}_{jt}",
                                        tag="ah1")
                aT0 = a_pool.tile([P, NT], BF16, name=f"aT0_{g}_{jt}", tag="aT")
                aT1 = a_pool.tile([P, NT], BF16, name=f"aT1_{g}_{jt}", tag="aT")
                nc.scalar.activation(out=aT0[:], in_=s0[:], func=AF.Exp)
                nc.scalar.activation(out=aT1[:], in_=s1[:], func=AF.Exp)
                ah0[jt] = a_pool.tile([P, NT], BF16, name=f"ab0_{g}_{jt}",
                                      tag="ab", bufs=6)
                nc.vector.tensor_tensor(out=ah0[jt][:], in0=aT0[:],
                                        in1=estr_tiles[h0][:, off : off + NT],
                                        op=OP.mult)
                nc.vector.tensor_tensor(out=ah1[jt][:], in0=aT1[:],
                                        in1=estr_tiles[h1][:, off : off + NT],
                                        op=OP.mult)
                # attn@V for h0, also two jt iterations behind on the
                # stored aT tile so these MMs never wait on DVE either
                if jt >= 2:
                    for ic in range(IC):
                        sl = slice(ic * 512, ic * 512 + 512)
                        nc.tensor.matmul(
                            o0[:, sl],
                            lhsT=v_sb[:, jt - 2,
                                      h0 * (HD + 1) : (h0 + 1) * (HD + 1)],
                            rhs=ah0[jt - 2][:, sl],
                            start=(jt == 2), stop=False)
                # attn@V for h1, two jt iterations behind (its aT tiles
                # are stored, so these MMs never wait on DVE)
                if jt >= 2:
                    for ic in range(IC):
                        sl = slice(ic * 512, ic * 512 + 512)
                        nc.tensor.matmul(
                            o1[:, sl],
                            lhsT=v_sb[:, jt - 2,
                                      h1 * (HD + 1) : (h1 + 1) * (HD + 1)],
                            rhs=ah1[jt - 2][:, sl],
                            start=(jt == 2), stop=False)
                # PE gap-filler: one Q/K chunk of the next pair per odd jt
                if jt % 2 == 1 and qk_fill:
                    emit_qk(*qk_fill.pop(0))

            # drain the lagged accumulations (jt 6, 7, both heads)
            for jd in (TT - 2, TT - 1):
                for ic in range(IC):
                    sl = slice(ic * 512, ic * 512 + 512)
                    nc.tensor.matmul(
                        o0[:, sl],
                        lhsT=v_sb[:, jd, h0 * (HD + 1) : (h0 + 1) * (HD + 1)],
                        rhs=ah0[jd][:, sl],
                        start=False, stop=(jd == TT - 1))
            for jd in (TT - 2, TT - 1):
                for ic in range(IC):
                    sl = slice(ic * 512, ic * 512 + 512)
                    nc.tensor.matmul(
                        o1[:, sl],
                        lhsT=v_sb[:, jd, h1 * (HD + 1) : (h1 + 1) * (HD + 1)],
                        rhs=ah1[jd][:, sl],
                        start=False, stop=(jd == TT - 1))

            # evacuate h0: unnormalized oT (bf16) + Z row
            ob0 = work.tile([HD, NT], BF16, name=f"ob0_{g}", tag="ob")
            nc.scalar.activation(out=ob0[:], in_=o0[:HD, :], func=AF.Copy)
            nc.vector.tensor_copy(out=zrowZ_sb[HD : HD + 1, 0, :],
                                  in_=o0[HD : HD + 1, :])
            nc.gpsimd.dma_start(
                zsm_sb[:, 0, :],
                zrowZ_sb[HD : HD + 1, 0, :].rearrange("o (p j) -> o p j", j=8))

            # ---- z-chain: 1/Z rows via DMA-redistribution ----
            # DMA Z into a (128, 16) layout (DVE recip is ~6 cycles per FREE
            # element, so keep the free dim tiny), recip, cast bf16, DMA
            # back to partition-0 rows for the K=1 replicate matmuls
            ob1 = work.tile([HD, NT], BF16, name=f"ob1_{g}", tag="ob")
            nc.scalar.activation(out=ob1[:], in_=o1[:HD, :], func=AF.Copy)
            nc.vector.tensor_copy(out=zrowZ_sb[HD : HD + 1, 1, :],
                                  in_=o1[HD : HD + 1, :])
            nc.gpsimd.dma_start(
                zsm_sb[:, 1, :],
                zrowZ_sb[HD : HD + 1, 1, :].rearrange("o (p j) -> o p j", j=8))
            nc.vector.reciprocal(out=zsm_sb[:], in_=zsm_sb[:])
            nc.vector.tensor_copy(out=zrb_sb[:], in_=zsm_sb[:])
            for e in range(2):
                nc.gpsimd.dma_start(
                    zrow_sb[:, e, :].rearrange("o (p j) -> o p j", j=8),
                    zrb_sb[:, e, :])
            pend = (g, ob0, ob1)

        # HAM filler across pair-3's z-chain so proj runs at full clock
        warm3 = psM.tile([P, NT], F32, name="warm3", tag="s")
        for i in range(40):
            nc.tensor.matmul(warm3[:, :P], lhsT=ones_mat[:], rhs=ones_mat[:],
                             start=True, stop=True, skip_group_check=True)
        emit_znorm(*pend)

        # ---------- phase 4: output projection + residual ----------
        for ct in range(CT):
            csl = slice(ct * P, ct * P + P)
            for ic in range(IC):
                sl = slice(ic * 512, ic * 512 + 512)
                y_ps = psM.tile([P, NT // 2], F32, name=f"y_ps_{ct}_{ic}",
                                tag="s")
                for g in range(NP):
                    nc.tensor.matmul(y_ps[:], lhsT=wpP_sb[:, g, csl],
                                     rhs=oTP_sb[:, g, sl],
                                     start=(g == 0), stop=(g == NP - 1))
                y_sb = work.tile([P, 512], F32, name=f"y_{ct}_{ic}", tag="y")
                nc.vector.scalar_tensor_tensor(
                    out=y_sb[:], in0=y_ps[:], scalar=bp_sb[:, ct : ct + 1],
                    in1=x_sb[:, ct, sl], op0=OP.add, op1=OP.add)
                nc.sync.dma_start(y_d[csl, sl], y_sb[:])

    return nc


def _legalize_waits(nc, max_waits: int = 1):
    """Split multi-wait instructions into preceding same-engine NoOps.

    The TPB instruction encoding carries a single sync-wait slot and this
    walrus build refuses to legalize ("Too many sync wait commands"), so do
    it here: engines execute their queue in order, so a NoOp carrying one of
    the waits delays everything after it on that engine identically.
    """
    import orjson

    data = orjson.loads(mybir.module_to_json_bytes(nc.m))
    ctr = [0]

    def fix_block(block):
        out = []
        for inst in block.get("instructions", []):
            si = inst.get("sync_info") or {}
            waits = si.get("on_wait") or []
            if len(waits) > max_waits:
                for w in waits[max_waits:]:
                    ctr[0] += 1
                    nop = {
                        "name": f"I-WS{ctr[0]}",
                        "opcode": "NoOp",
                        "engine": inst["engine"],
                        "ins": [],
                        "outs": [],
                        "sync_info": {"on_wait": [w], "on_update": []},
                    }
                    if "debug" in inst:
                        nop["debug"] = inst["debug"]
                    out.append(nop)
                si = dict(si)
                si["on_wait"] = waits[:max_waits]
                inst["sync_info"] = si
            out.append(inst)
        block["instructions"] = out
        for b in block.get("blocks", []):
            fix_block(b)

    for fn in data["functions"]:
        for b in fn.get("blocks", []):
            fix_block(b)
    nc.m = mybir.module_from_json_bytes(orjson.dumps(data))
    return nc


_NC = None


def _host_prep(x, norm_w, norm_b, wq, bq, wk, bk, wv, bv, wp, bp, rel):
    scale = HD ** -0.5
    BF = ml_dtypes.bfloat16
    # fold LN affine + score scale into the projection weights (exact algebra)
    wq_eff = (wq * norm_w[None, :]) * scale
    bq_eff = (bq + wq @ norm_b) * scale
    wk_eff = wk * norm_w[None, :]
    bk_eff = bk + wk @ norm_b
    wv_eff = wv * norm_w[None, :]
    bv_eff = bv + wv @ norm_b

    wqT = np.ascontiguousarray(wq_eff.T).astype(BF)
    wkT = np.ascontiguousarray(wk_eff.T).astype(BF)
    wvT = np.ascontiguousarray(wv_eff.T).astype(BF)
    # wp pair-stacked: wpP[p, g, c] = wp.T[(2g + p//64)*64 + p%64, c]
    wpP = np.ascontiguousarray(
        wp.T.reshape(NP, P, CH)
    ).transpose(1, 0, 2)
    wpP = np.ascontiguousarray(wpP).astype(BF)

    bqk = np.stack([bq_eff.reshape(CT, P).T, bk_eff.reshape(CT, P).T],
                   axis=1).astype(np.float32)          # (P, 2, CT)
    bp_a = np.ascontiguousarray(bp.reshape(CT, P).T).astype(np.float32)  # (P, CT)
    bv_a = bv_eff.reshape(1, CH).astype(BF)
    estr = np.exp(_build_strips(np.asarray(rel, np.float32))).astype(BF)

    shared = {
        "wqT": wqT, "wkT": wkT, "wvT": wvT, "wpP": wpP,
        "bqk": bqk, "bp": bp_a, "bv": bv_a, "estr": estr,
    }
    in_maps = []
    for b in range(B):
        m = dict(shared)
        m["x"] = np.ascontiguousarray(x[b].reshape(CH, NT)).astype(np.float32)
        in_maps.append(m)
    return in_maps


def kernel(**inputs):
    global _NC
    if _NC is None:
        _NC = _legalize_waits(_build_nc())
    in_maps = _host_prep(**{k: np.asarray(v) for k, v in inputs.items()})
    res = run_bass_kernel_spmd(_NC, in_maps, list(range(B)))
    out = np.stack([res.results[b]["y"].reshape(CH, H, W) for b in range(B)])
    return out.astype(np.float32)


if __name__ == "__main__":
    nc = _build_nc()
    print("built OK")


# revision 70
# speedup vs baseline: 1.0019x; 1.0019x over previous
"""Trainium2 Bass kernel for nn_MHAAttention (LayerNorm2d + MHA w/ rel-pos bias + residual).

Sharding: data-parallel over batch - 8 batch elements, one per NeuronCore.
No collectives needed. ~3x faster than the fp32 baseline (632us -> ~211us).

Design notes (all matmuls bf16: 1 cycle/row on the PE vs fp32's 4):
  - LN folded into the projection weights (exact algebra); stats via
    ones-matmuls on a bf16 cast of x; the per-token 1/sqrt(var) and -mu*rs
    are computed on a DMA-redistributed (128, 8) layout because the DVE
    reciprocal costs ~6 cycles per FREE element, then replicated across
    partitions with K=1 ones-matmuls.
  - scores computed per head-pair via PE row-tiling: head 2g's K=64
    contraction on array rows 0-63, head 2g+1 on rows 64-127 — the two
    matmul streams partially overlap on the 128x128 array.
  - rel-pos bias applied multiplicatively AFTER exp: aT = exp(s) * expstrip
    (expstrip = exp(bias strip), host-precomputed bf16, block-Toeplitz
    compressed to a (128, 1920) strip per head) so the DVE op runs in the
    2x bf16 mode instead of a 1x fp32 add against PSUM.
  - softmax denominator Z from a ones-augmented V column; 1/Z also via the
    DMA-redistribute-small trick; normalization pipelined one pair late so
    the DMA round trip hides behind PE work.
  - attn@V for the even head streams behind the exps; for the odd head it
    runs 2 jt-iterations behind on stored aT tiles, so the PE queue never
    waits on the vector engines.
  - Q/K projections of the NEXT pair and dummy warm-up matmuls are
    interleaved as PE gap-filler: the HAM clock gate re-throttles the PE
    array to 1.2 GHz after any idle window, so the stream must stay dense.
  - output projection with K=128 head-pair stacking; odd-head oT shifted to
    partitions 64-127 via SBUF-to-SBUF DMA.
"""

import sys

for _p in ("/opt/trn_rl_repo",):
    if _p not in sys.path:
        sys.path.insert(0, _p)

from contextlib import ExitStack

import numpy as np
import ml_dtypes

import concourse.bass as bass
import concourse.mybir as mybir
import concourse.tile as tile
from concourse.bass_utils import run_bass_kernel_spmd

F32 = mybir.dt.float32
F32R = mybir.dt.float32r
BF16 = mybir.dt.bfloat16
AF = mybir.ActivationFunctionType
OP = mybir.AluOpType

B = 8
CH = 512
H = W = 32
NT = H * W          # 1024 tokens
HEADS = 8
HD = 64
EPS = 1e-6
P = 128
CT = CH // P        # 4 channel tiles
TT = NT // P        # 8 token tiles
IC = NT // 512      # 2 free-dim chunks of 512
NP = HEADS // 2     # 4 head pairs
STRIP_W = 60 * 32   # 1920


def _build_strips(rel: np.ndarray) -> np.ndarray:
    """(3969, 8) rel table -> (8, 128, 1920) bias strips.

    strip[h, 32*jh_l + jw, 32*g + iw] = T_h[g - jh_l + 3, iw - jw + 31]
    where T_h = rel[:, h].reshape(63, 63).
    bias.T block for key-tile jt is then strip[:, (28-4*jt)*32 : +1024].
    """
    T = rel.reshape(63, 63, HEADS)  # [a, b, h]
    jh_l = np.arange(4)[:, None, None, None]
    jw = np.arange(32)[None, :, None, None]
    g = np.arange(60)[None, None, :, None]
    iw = np.arange(32)[None, None, None, :]
    a = g - jh_l + 3          # in [0,62]
    b = iw - jw + 31          # in [0,62]
    a_b, b_b = np.broadcast_arrays(a, b)
    out = T[a_b, b_b, :]      # (4, 32, 60, 32, 8)
    out = np.ascontiguousarray(np.moveaxis(out, -1, 0)).reshape(HEADS, P, STRIP_W)
    return out.astype(np.float32)


def _build_nc() -> bass.Bass:
    nc = bass.Bass()

    x_d = nc.declare_dram_parameter("x", [CH, NT], F32, isOutput=False)
    wqT_d = nc.declare_dram_parameter("wqT", [CH, CH], BF16, isOutput=False)
    wkT_d = nc.declare_dram_parameter("wkT", [CH, CH], BF16, isOutput=False)
    wvT_d = nc.declare_dram_parameter("wvT", [CH, CH], BF16, isOutput=False)
    wpP_d = nc.declare_dram_parameter("wpP", [P, NP, CH], BF16, isOutput=False)
    bqk_d = nc.declare_dram_parameter("bqk", [P, 2, CT], F32, isOutput=False)
    bp_d = nc.declare_dram_parameter("bp", [P, CT], F32, isOutput=False)
    bv_d = nc.declare_dram_parameter("bv", [1, CH], BF16, isOutput=False)
    estr_d = nc.declare_dram_parameter("estr", [HEADS, P, STRIP_W], BF16, isOutput=False)
    y_d = nc.declare_dram_parameter("y", [CH, NT], F32, isOutput=True)

    with tile.TileContext(nc) as tc, ExitStack() as ctx:
        singles = ctx.enter_context(tc.tile_pool(name="singles", bufs=1))
        work = ctx.enter_context(tc.tile_pool(name="work", bufs=2))
        strip_pool = ctx.enter_context(tc.tile_pool(name="strip_pool", bufs=4))
        a_pool = ctx.enter_context(tc.tile_pool(name="a_pool", bufs=4))
        ah1_pool = ctx.enter_context(tc.tile_pool(name="ah1_pool", bufs=6))
        # PSUM: psM 2 slots x (128,1024)f32 = 4 banks; psO 2 x (65,1024) = 4.
        psM = ctx.enter_context(tc.tile_pool(name="psM", bufs=2, space="PSUM"))
        psO = ctx.enter_context(tc.tile_pool(name="psO", bufs=2, space="PSUM"))

        # ---------- persistent SBUF ----------
        x_sb = singles.tile([P, CT, NT], F32)        # raw x (residual + LN apply)
        xb_sb = singles.tile([P, CT, NT], BF16)      # LN output (normalized bf16)
        qT_sb = singles.tile([P, CT, NT], BF16)      # (d part, t free), pair-stacked
        kT_sb = singles.tile([P, CT, NT], BF16)
        v_sb = singles.tile([P, TT, HEADS * (HD + 1)], BF16)  # per head [v(64)|1]
        oTP_sb = singles.tile([P, NP, NT], BF16)     # pair-stacked normalized oT
        wpP_sb = singles.tile([P, NP, CH], BF16)
        bqk_sb = singles.tile([P, 2, CT], F32)
        bp_sb = singles.tile([P, CT], F32)
        bv_sb = singles.tile([1, CH], BF16)
        bvb_sb = singles.tile([P, CH], BF16)         # bv broadcast across partitions
        ones_mat = singles.tile([P, P], BF16)
        ones_row = singles.tile([1, P], BF16)
        eps_sb = singles.tile([P, 1], F32)
        nc.vector.memset(eps_sb[:], float(EPS))
        lnsm_sb = singles.tile([P, 2, 8], F32)       # LN mu/ve small layout
        lnsmb_sb = singles.tile([P, 2, 8], BF16)     # LN rs,b small bf16
        lnrow_sb = singles.tile([1, 2, NT], BF16)    # rs,b rows at partition 0
        lnmv_sb = singles.tile([1, 2, NT], F32)      # mu,ve rows at partition 0
        zrowZ_sb = singles.tile([HD + 1, 2, NT], F32)  # Z rows evac'd at part. 64
        zsm_sb = singles.tile([P, 2, 8], F32)        # Z small layout (per pair)
        zrb_sb = singles.tile([P, 2, 8], BF16)       # 1/Z small, bf16
        zrow_sb = singles.tile([1, 2, NT], BF16)     # 1/Z rows at partition 0

        # x first, in per-ct chunks — the LN cast chases the chunks
        x_r = x_d.rearrange("(ct p) t -> ct p t", p=P)
        for ct in range(CT):
            nc.sync.dma_start(x_sb[:, ct], x_r[ct])
        nc.vector.memset(ones_mat[:], 1.0)
        nc.vector.memset(ones_row[:], 1.0)
        nc.sync.dma_start(bqk_sb[:], bqk_d[:])
        nc.sync.dma_start(bp_sb[:], bp_d[:])
        nc.sync.dma_start(bv_sb[:], bv_d[:])
        nc.sync.dma_start(wpP_sb[:], wpP_d[:])

        # ones columns of v_aug
        v_view = v_sb[:].rearrange("p tt (h w) -> p tt h w", w=HD + 1)
        nc.vector.memset(v_view[:, :, :, HD : HD + 1], 1.0)

        wqT_sb = singles.tile([P, CT, CH], BF16)
        wkT_sb = singles.tile([P, CT, CH], BF16)
        wvT_sb = singles.tile([P, CT, CH], BF16)
        nc.sync.dma_start(wqT_sb[:], wqT_d.rearrange("(ck p) d -> p ck d", p=P))
        nc.sync.dma_start(wkT_sb[:], wkT_d.rearrange("(ck p) d -> p ck d", p=P))
        nc.sync.dma_start(wvT_sb[:], wvT_d.rearrange("(ck p) d -> p ck d", p=P))

        # strips for pairs 0,1 DMA'd in the prologue; pairs 2,3 prefetched
        # from inside the pair loop (keeps the sync queue from stalling on
        # the strip-slot WAR semaphore ahead of the z-chain DMAs)
        estr_tiles = []
        for h in range(HEADS):
            st = strip_pool.tile([P, STRIP_W], BF16, name=f"estr_{h}", tag="strip")
            if h < 4:
                nc.sync.dma_start(st[:], estr_d[h])
            estr_tiles.append(st)

        # PE warmup: dummy matmuls during the x DMA (HAM un-throttle needs
        # ~3.4us of sustained PE activity; these overlap the input DMA).
        warm_ps = psM.tile([P, NT], F32, tag="s")
        for i in range(16):
            nc.tensor.matmul(warm_ps[:, :P], lhsT=ones_mat[:], rhs=ones_mat[:],
                             start=True, stop=True, skip_group_check=True)
        # bv broadcast across partitions (K=1 ones-column matmul + ACT evac)
        nc.tensor.matmul(warm_ps[:, :CH], lhsT=ones_row[:], rhs=bv_sb[:],
                         start=True, stop=True, skip_group_check=True)
        nc.scalar.activation(out=bvb_sb[:], in_=warm_ps[:, :CH], func=AF.Copy)

        # ---------- phase 1: LayerNorm ----------
        # bf16 cast of raw x (DVE, pipelined with the x DMA) + squares on
        # ACT (idle in this phase); stats via bf16 ones-matmuls
        with tc.tile_pool(name="ln_pool", bufs=2) as lnp, \
             tc.tile_pool(name="ln_single", bufs=1) as lns:
            # xb_sb temporarily holds the raw-x bf16 cast (overwritten by the
            # normalized output after the stats matmuls complete); cast,
            # square, and stats matmuls interleaved per channel tile
            sum_ps = psM.tile([P, NT], F32, tag="s")
            sq_ps = psM.tile([P, NT], F32, tag="s")
            for ct in range(CT):
                nc.vector.tensor_copy(out=xb_sb[:, ct], in_=x_sb[:, ct])
                x2 = lnp.tile([P, NT], BF16, name=f"x2_{ct}", tag="x2", bufs=2)
                nc.scalar.activation(out=x2[:], in_=xb_sb[:, ct], func=AF.Square)
                for ic in range(IC):
                    sl = slice(ic * 512, ic * 512 + 512)
                    nc.tensor.matmul(sum_ps[:, sl], lhsT=ones_mat[:],
                                     rhs=xb_sb[:, ct, sl],
                                     start=(ct == 0), stop=(ct == CT - 1))
                    nc.tensor.matmul(sq_ps[:, sl], lhsT=ones_mat[:],
                                     rhs=x2[:, sl],
                                     start=(ct == 0), stop=(ct == CT - 1))

            # LN scalars on a DMA-redistributed (128, 8) small layout: the
            # replicated rows of sum/sq go through ACT copies, one row is
            # DMA'd small, rs = 1/sqrt(var+eps) and b = -mu*rs cost ~100ns
            # each there (DVE recip is ~6 cyc per FREE elem), then rows are
            # DMA'd back and partition-broadcast by a stride-0 DMA.
            rs_bc = lns.tile([P, NT], F32)
            b_bc = lns.tile([P, NT], F32)
            nc.scalar.activation(out=lnmv_sb[:, 0, :], in_=sum_ps[0:1, :],
                                 func=AF.Copy, scale=1.0 / CH)
            nc.scalar.activation(out=lnmv_sb[:, 1, :], in_=sq_ps[0:1, :],
                                 func=AF.Copy, scale=1.0 / CH)
            for e in range(2):
                nc.scalar.dma_start(
                    lnsm_sb[:, e, :],
                    lnmv_sb[:, e, :].rearrange("o (p j) -> o p j", j=8))
            musq = lns.tile([P, 8], F32)
            var_s = lns.tile([P, 8], F32)
            rs_s = lns.tile([P, 8], F32)
            b_s = lns.tile([P, 8], F32)
            nc.vector.tensor_tensor(out=musq[:], in0=lnsm_sb[:, 0, :],
                                    in1=lnsm_sb[:, 0, :], op=OP.mult)
            nc.vector.tensor_tensor(out=var_s[:], in0=lnsm_sb[:, 1, :],
                                    in1=musq[:], op=OP.subtract)
            nc.scalar.activation(out=var_s[:], in_=var_s[:], func=AF.Sqrt,
                                 bias=eps_sb[:])
            nc.vector.reciprocal(out=rs_s[:], in_=var_s[:])
            nc.vector.scalar_tensor_tensor(out=b_s[:], in0=lnsm_sb[:, 0, :],
                                           scalar=-1.0, in1=rs_s[:],
                                           op0=OP.mult, op1=OP.mult)
            nc.vector.tensor_copy(out=lnsmb_sb[:, 0, :], in_=rs_s[:])
            nc.vector.tensor_copy(out=lnsmb_sb[:, 1, :], in_=b_s[:])
            for e in range(2):
                nc.gpsimd.dma_start(
                    lnrow_sb[:, e, :].rearrange("o (p j) -> o p j", j=8),
                    lnsmb_sb[:, e, :])
            # HAM filler: keep the PE array active across the LN small-DMA
            # chain so the V/QK phases start at full clock
            warm2 = psM.tile([P, NT], F32, name="warm2", tag="s")
            for i in range(28):
                nc.tensor.matmul(warm2[:, :P], lhsT=ones_mat[:], rhs=ones_mat[:],
                                 start=True, stop=True, skip_group_check=True)
            # replicate rs,b across partitions: K=1 matmuls + ACT evacuation
            rep_ps = psM.tile([P, NT], F32, name="lnrep_ps", tag="s")
            bep_ps = psM.tile([P, NT], F32, name="lnbep_ps", tag="s")
            for ic in range(IC):
                sl = slice(ic * 512, ic * 512 + 512)
                nc.tensor.matmul(rep_ps[:, sl], lhsT=ones_mat[0:1, :],
                                 rhs=lnrow_sb[:, 0, sl], start=True, stop=True)
                nc.tensor.matmul(bep_ps[:, sl], lhsT=ones_mat[0:1, :],
                                 rhs=lnrow_sb[:, 1, sl], start=True, stop=True)
            nc.scalar.activation(out=rs_bc[:], in_=rep_ps[:], func=AF.Copy)
            nc.scalar.activation(out=b_bc[:], in_=bep_ps[:], func=AF.Copy)

            # apply xb = x*rs + b, split across DVE and GPSIMD (crosswise so
            # each ct's chain spans both engines and they run in parallel)
            for ct in range(CT):
                xm = lnp.tile([P, NT], F32, name=f"xm_{ct}", tag="xm", bufs=2)
                e_mul = nc.vector if ct < 2 else nc.gpsimd
                e_add = nc.gpsimd if ct < 2 else nc.vector
                e_mul.tensor_tensor(out=xm[:], in0=x_sb[:, ct], in1=rs_bc[:],
                                    op=OP.mult)
                e_add.tensor_tensor(out=xb_sb[:, ct], in0=xm[:], in1=b_bc[:],
                                    op=OP.add)

        # ---------- phase 2a: V projection ----------
        # token tiles 0,1 up front; 2..7 interleaved into pair 0's jt loop
        # (two iterations ahead of their first consumer) to absorb the
        # otherwise-serial V phase into the attention pipeline
        def emit_v(tt):
            tsl = slice(tt * P, tt * P + P)
            v_ps = psM.tile([P, NT], F32, name=f"v_ps_{tt}", tag="s")
            for ck in range(CT):
                nc.tensor.matmul(v_ps[:, :512], lhsT=xb_sb[:, ck, tsl],
                                 rhs=wvT_sb[:, ck, :],
                                 start=(ck == 0), stop=(ck == CT - 1))
            vp_v = v_ps[:, :512].rearrange("p (h w) -> p h w", w=HD)
            bv_v = bvb_sb[:].rearrange("p (h w) -> p h w", w=HD)
            nc.vector.scalar_tensor_tensor(
                out=v_view[:, tt, :, :HD], in0=vp_v, scalar=0.0,
                in1=bv_v, op0=OP.bypass, op1=OP.add)

        emit_v(0)
        emit_v(1)

        # ---------- phases 2b+3: per head pair: Q/K proj then attention ----------
        def emit_znorm(g, ob0, ob1):
            """Replicate 1/Z rows (K=1 matmuls) and normalize into oTP.

            Emitted one pair LATE (mid next pair's jt loop) so the z-chain's
            DMA round-trip latency is hidden behind PE work instead of
            stalling the in-order PE queue.
            """
            zr0 = psM.tile([P, NT], F32, name=f"zr0_{g}", tag="s")
            zr1 = psM.tile([P, NT], F32, name=f"zr1_{g}", tag="s")
            for ic in range(IC):
                sl = slice(ic * 512, ic * 512 + 512)
                nc.tensor.matmul(zr0[:HD, sl], lhsT=ones_mat[0:1, :HD],
                                 rhs=zrow_sb[:, 0, sl], start=True, stop=True)
                nc.tensor.matmul(zr1[:HD, sl], lhsT=ones_mat[0:1, :HD],
                                 rhs=zrow_sb[:, 1, sl], start=True, stop=True)
            nc.vector.tensor_tensor(out=oTP_sb[:HD, g], in0=ob0[:],
                                    in1=zr0[:HD, :], op=OP.mult)
            tmpO = work.tile([HD, NT], BF16, name=f"tmpO_{g}", tag="tmpO")
            nc.vector.tensor_tensor(out=tmpO[:], in0=ob1[:],
                                    in1=zr1[:HD, :], op=OP.mult)
            nc.gpsimd.dma_start(oTP_sb[HD:, g], tmpO[:])

        def emit_qk(gq, which, ic):
            """One Q or K projection chunk (4-MM accumulation + evac)."""
            sl = slice(ic * 512, ic * 512 + 512)
            dq = slice(gq * P, gq * P + P)
            w_sb = wqT_sb if which == 0 else wkT_sb
            dst = qT_sb if which == 0 else kT_sb
            ps = psM.tile([P, NT], F32, name=f"qk_ps_{gq}_{which}_{ic}", tag="s")
            for ck in range(CT):
                nc.tensor.matmul(ps[:, :512], lhsT=w_sb[:, ck, dq],
                                 rhs=xb_sb[:, ck, sl],
                                 start=(ck == 0), stop=(ck == CT - 1))
            nc.vector.tensor_scalar_add(out=dst[:, gq, sl], in0=ps[:, :512],
                                        scalar1=bqk_sb[:, which, gq : gq + 1])

        # Q/K for pair 0 up front
        for ic in range(IC):
            emit_qk(0, 0, ic)
            emit_qk(0, 1, ic)

        pend = None
        for g in range(NP):
            h0, h1 = 2 * g, 2 * g + 1
            # prefetch strips for pair g+2
            if g < 2:
                nc.sync.dma_start(estr_tiles[2 * g + 4][:], estr_d[2 * g + 4])
                nc.sync.dma_start(estr_tiles[2 * g + 5][:], estr_d[2 * g + 5])
            # Q/K chunks for pair g+1, interleaved into this pair's jt loop
            # (fills PE stall slivers so the HAM clock gate stays warm)
            qk_fill = ([(g + 1, w, ic) for ic in range(IC) for w in (0, 1)]
                       if g < NP - 1 else [])

            # attention for heads (h0: partitions 0-63, h1: 64-127)
            o0 = psO.tile([HD + 1, NT], F32, name=f"o0_{g}", tag="o")
            o1 = psO.tile([HD + 1, NT], F32, name=f"o1_{g}", tag="o")
            ah0 = {}
            ah1 = {}
            for jt in range(TT):
                jsl = slice(jt * P, jt * P + P)
                off = (28 - 4 * jt) * 32
                if g == 0 and jt < TT - 2:
                    emit_v(jt + 2)
                # pending z-normalization of the PREVIOUS pair (its DMA
                # chain has had ~3 jt iterations of PE work to complete)
                if jt == 3 and pend is not None:
                    emit_znorm(*pend)
                    pend = None
                # paired score matmuls: (0,0) and (64,0) row tiles run
                # concurrently on the PE array
                s0 = psM.tile([P, NT], F32, name=f"s0_{g}_{jt}", tag="s")
                s1 = psM.tile([P, NT], F32, name=f"s1_{g}_{jt}", tag="s")
                # grouped per head (A0,A1 then B0,B1): each head's lhsT loads
                # once and head B's LDWEIGHTS pulls ahead during A's MMs
                # (different row groups); a fully alternating order measured
                # ~35us WORSE (per-MM weight reloads, no pull-ahead)
                for ic in range(IC):
                    sl = slice(ic * 512, ic * 512 + 512)
                    nc.tensor.matmul(s0[:, sl], lhsT=kT_sb[:HD, g, jsl],
                                     rhs=qT_sb[:HD, g, sl], start=True, stop=True)
                for ic in range(IC):
                    sl = slice(ic * 512, ic * 512 + 512)
                    nc.tensor.matmul(s1[:, sl], lhsT=kT_sb[HD:, g, jsl],
                                     rhs=qT_sb[HD:, g, sl], start=True, stop=True)
                ah1[jt] = ah1_pool.tile([P, NT], BF16, name=f"ah1_{g}_{jt}",
                                        tag="ah1")
                aT0 = a_pool.tile([P, NT], BF16, name=f"aT0_{g}_{jt}", tag="aT")
                aT1 = a_pool.tile([P, NT], BF16, name=f"aT1_{g}_{jt}", tag="aT")
                nc.scalar.activation(out=aT0[:], in_=s0[:], func=AF.Exp)
                nc.scalar.activation(out=aT1[:], in_=s1[:], func=AF.Exp)
                ah0[jt] = a_pool.tile([P, NT], BF16, name=f"ab0_{g}_{jt}",
                                      tag="ab", bufs=6)
                nc.vector.tensor_tensor(out=ah0[jt][:], in0=aT0[:],
                                        in1=estr_tiles[h0][:, off : off + NT],
                                        op=OP.mult)
                nc.vector.tensor_tensor(out=ah1[jt][:], in0=aT1[:],
                                        in1=estr_tiles[h1][:, off : off + NT],
                                        op=OP.mult)
                # attn@V for h0, also two jt iterations behind on the
                # stored aT tile so these MMs never wait on DVE either
                if jt >= 2:
                    for ic in range(IC):
                        sl = slice(ic * 512, ic * 512 + 512)
                        nc.tensor.matmul(
                            o0[:, sl],
                            lhsT=v_sb[:, jt - 2,
                                      h0 * (HD + 1) : (h0 + 1) * (HD + 1)],
                            rhs=ah0[jt - 2][:, sl],
                            start=(jt == 2), stop=False)
                # attn@V for h1, two jt iterations behind (its aT tiles
                # are stored, so these MMs never wait on DVE)
                if jt >= 2:
                    for ic in range(IC):
                        sl = slice(ic * 512, ic * 512 + 512)
                        nc.tensor.matmul(
                            o1[:, sl],
                            lhsT=v_sb[:, jt - 2,
                                      h1 * (HD + 1) : (h1 + 1) * (HD + 1)],
                            rhs=ah1[jt - 2][:, sl],
                            start=(jt == 2), stop=False)
                # PE gap-filler: one Q/K chunk of the next pair per odd jt
                if jt % 2 == 1 and qk_fill:
                    emit_qk(*qk_fill.pop(0))

            # drain the lagged accumulations (jt 6, 7, both heads)
            for jd in (TT - 2, TT - 1):
                for ic in range(IC):
                    sl = slice(ic * 512, ic * 512 + 512)
                    nc.tensor.matmul(
                        o0[:, sl],
                        lhsT=v_sb[:, jd, h0 * (HD + 1) : (h0 + 1) * (HD + 1)],
                        rhs=ah0[jd][:, sl],
                        start=False, stop=(jd == TT - 1))
            for jd in (TT - 2, TT - 1):
                for ic in range(IC):
                    sl = slice(ic * 512, ic * 512 + 512)
                    nc.tensor.matmul(
                        o1[:, sl],
                        lhsT=v_sb[:, jd, h1 * (HD + 1) : (h1 + 1) * (HD + 1)],
                        rhs=ah1[jd][:, sl],
                        start=False, stop=(jd == TT - 1))

            # evacuate h0: unnormalized oT (bf16) + Z row
            ob0 = work.tile([HD, NT], BF16, name=f"ob0_{g}", tag="ob")
            nc.scalar.activation(out=ob0[:], in_=o0[:HD, :], func=AF.Copy)
            nc.vector.tensor_copy(out=zrowZ_sb[HD : HD + 1, 0, :],
                                  in_=o0[HD : HD + 1, :])
            nc.gpsimd.dma_start(
                zsm_sb[:, 0, :],
                zrowZ_sb[HD : HD + 1, 0, :].rearrange("o (p j) -> o p j", j=8))

            # ---- z-chain: 1/Z rows via DMA-redistribution ----
            # DMA Z into a (128, 16) layout (DVE recip is ~6 cycles per FREE
            # element, so keep the free dim tiny), recip, cast bf16, DMA
            # back to partition-0 rows for the K=1 replicate matmuls
            ob1 = work.tile([HD, NT], BF16, name=f"ob1_{g}", tag="ob")
            nc.scalar.activation(out=ob1[:], in_=o1[:HD, :], func=AF.Copy)
            nc.vector.tensor_copy(out=zrowZ_sb[HD : HD + 1, 1, :],
                                  in_=o1[HD : HD + 1, :])
            nc.gpsimd.dma_start(
                zsm_sb[:, 1, :],
                zrowZ_sb[HD : HD + 1, 1, :].rearrange("o (p j) -> o p j", j=8))
            nc.vector.reciprocal(out=zsm_sb[:], in_=zsm_sb[:])
            nc.vector.tensor_copy(out=zrb_sb[:], in_=zsm_sb[:])
            for e in range(2):
                nc.gpsimd.dma_start(
                    zrow_sb[:, e, :].rearrange("o (p j) -> o p j", j=8),
                    zrb_sb[:, e, :])
            pend = (g, ob0, ob1)

        # HAM filler across pair-3's z-chain so proj runs at full clock
        warm3 = psM.tile([P, NT], F32, name="warm3", tag="s")
        for i in range(12):
            nc.tensor.matmul(warm3[:, :P], lhsT=ones_mat[:], rhs=ones_mat[:],
                             start=True, stop=True, skip_group_check=True)
        emit_znorm(*pend)

        # ---------- phase 4: output projection + residual ----------
        for ct in range(CT):
            csl = slice(ct * P, ct * P + P)
            for ic in range(IC):
                sl = slice(ic * 512, ic * 512 + 512)
                y_ps = psM.tile([P, NT // 2], F32, name=f"y_ps_{ct}_{ic}",
                                tag="s")
                for g in range(NP):
                    nc.tensor.matmul(y_ps[:], lhsT=wpP_sb[:, g, csl],
                                     rhs=oTP_sb[:, g, sl],
                                     start=(g == 0), stop=(g == NP - 1))
                y_sb = work.tile([P, 512], F32, name=f"y_{ct}_{ic}", tag="y")
                nc.vector.scalar_tensor_tensor(
                    out=y_sb[:], in0=y_ps[:], scalar=bp_sb[:, ct : ct + 1],
                    in1=x_sb[:, ct, sl], op0=OP.add, op1=OP.add)
                nc.sync.dma_start(y_d[csl, sl], y_sb[:])

    return nc


def _legalize_waits(nc, max_waits: int = 1):
    """Split multi-wait instructions into preceding same-engine NoOps.

    The TPB instruction encoding carries a single sync-wait slot and this
    walrus build refuses to legalize ("Too many sync wait commands"), so do
    it here: engines execute their queue in order, so a NoOp carrying one of
    the waits delays everything after it on that engine identically.
    """
    import orjson

    data = orjson.loads(mybir.module_to_json_bytes(nc.m))
    ctr = [0]

    def fix_block(block):
        out = []
        for inst in block.get("instructions", []):
            si = inst.get("sync_info") or {}
            waits = si.get("on_wait") or []
            if len(waits) > max_waits:
                for w in waits[max_waits:]:
                    ctr[0] += 1
                    nop = {
                        "name": f"I-WS{ctr[0]}",
                        "opcode": "NoOp",
                        "engine": inst["engine"],
                        "ins": [],
                        "outs": [],
                        "sync_info": {"on_wait": [w], "on_update": []},
                    }
                    if "debug" in inst:
                        nop["debug"] = inst["debug"]
                    out.append(nop)
                si = dict(si)
                si["on_wait"] = waits[:max_waits]
                inst["sync_info"] = si
            out.append(inst)
        block["instructions"] = out
        for b in block.get("blocks", []):
            fix_block(b)

    for fn in data["functions"]:
        for b in fn.get("blocks", []):
            fix_block(b)
    nc.m = mybir.module_from_json_bytes(orjson.dumps(data))
    return nc


_NC = None


def _host_prep(x, norm_w, norm_b, wq, bq, wk, bk, wv, bv, wp, bp, rel):
    scale = HD ** -0.5
    BF = ml_dtypes.bfloat16
    # fold LN affine + score scale into the projection weights (exact algebra)
    wq_eff = (wq * norm_w[None, :]) * scale
    bq_eff = (bq + wq @ norm_b) * scale
    wk_eff = wk * norm_w[None, :]
    bk_eff = bk + wk @ norm_b
    wv_eff = wv * norm_w[None, :]
    bv_eff = bv + wv @ norm_b

    wqT = np.ascontiguousarray(wq_eff.T).astype(BF)
    wkT = np.ascontiguousarray(wk_eff.T).astype(BF)
    wvT = np.ascontiguousarray(wv_eff.T).astype(BF)
    # wp pair-stacked: wpP[p, g, c] = wp.T[(2g + p//64)*64 + p%64, c]
    wpP = np.ascontiguousarray(
        wp.T.reshape(NP, P, CH)
    ).transpose(1, 0, 2)
    wpP = np.ascontiguousarray(wpP).astype(BF)

    bqk = np.stack([bq_eff.reshape(CT, P).T, bk_eff.reshape(CT, P).T],
                   axis=1).astype(np.float32)          # (P, 2, CT)
    bp_a = np.ascontiguousarray(bp.reshape(CT, P).T).astype(np.float32)  # (P, CT)
    bv_a = bv_eff.reshape(1, CH).astype(BF)
    estr = np.exp(_build_strips(np.asarray(rel, np.float32))).astype(BF)

    shared = {
        "wqT": wqT, "wkT": wkT, "wvT": wvT, "wpP": wpP,
        "bqk": bqk, "bp": bp_a, "bv": bv_a, "estr": estr,
    }
    in_maps = []
    for b in range(B):
        m = dict(shared)
        m["x"] = np.ascontiguousarray(x[b].reshape(CH, NT)).astype(np.float32)
        in_maps.append(m)
    return in_maps


def kernel(**inputs):
    global _NC
    if _NC is None:
        _NC = _legalize_waits(_build_nc())
    in_maps = _host_prep(**{k: np.asarray(v) for k, v in inputs.items()})
    res = run_bass_kernel_spmd(_NC, in_maps, list(range(B)))
    out = np.stack([res.results[b]["y"].reshape(CH, H, W) for b in range(B)])
    return out.astype(np.float32)


if __name__ == "__main__":
    nc = _build_nc()
    print("built OK")


# revision 71
# speedup vs baseline: 1.1933x; 1.1910x over previous
"""Trainium2 Bass kernel for nn_MHAAttention (LayerNorm2d + MHA w/ rel-pos bias + residual).

Sharding: data-parallel over batch - 8 batch elements, one per NeuronCore.
No collectives needed. ~3x faster than the fp32 baseline (632us -> ~211us).

Design notes (all matmuls bf16: 1 cycle/row on the PE vs fp32's 4):
  - LN folded into the projection weights (exact algebra); stats via
    ones-matmuls on a bf16 cast of x; the per-token 1/sqrt(var) and -mu*rs
    are computed on a DMA-redistributed (128, 8) layout because the DVE
    reciprocal costs ~6 cycles per FREE element, then replicated across
    partitions with K=1 ones-matmuls.
  - scores computed per head-pair via PE row-tiling: head 2g's K=64
    contraction on array rows 0-63, head 2g+1 on rows 64-127 — the two
    matmul streams partially overlap on the 128x128 array.
  - rel-pos bias applied multiplicatively AFTER exp: aT = exp(s) * expstrip
    (expstrip = exp(bias strip), host-precomputed bf16, block-Toeplitz
    compressed to a (128, 1920) strip per head) so the DVE op runs in the
    2x bf16 mode instead of a 1x fp32 add against PSUM.
  - softmax denominator Z from a ones-augmented V column; 1/Z also via the
    DMA-redistribute-small trick; normalization pipelined one pair late so
    the DMA round trip hides behind PE work.
  - attn@V for the even head streams behind the exps; for the odd head it
    runs 2 jt-iterations behind on stored aT tiles, so the PE queue never
    waits on the vector engines.
  - Q/K projections of the NEXT pair and dummy warm-up matmuls are
    interleaved as PE gap-filler: the HAM clock gate re-throttles the PE
    array to 1.2 GHz after any idle window, so the stream must stay dense.
  - output projection with K=128 head-pair stacking; odd-head oT shifted to
    partitions 64-127 via SBUF-to-SBUF DMA.
"""

import sys

for _p in ("/opt/trn_rl_repo",):
    if _p not in sys.path:
        sys.path.insert(0, _p)

from contextlib import ExitStack

import numpy as np
import ml_dtypes

import concourse.bass as bass
import concourse.mybir as mybir
import concourse.tile as tile
from concourse.bass_utils import run_bass_kernel_spmd

F32 = mybir.dt.float32
F32R = mybir.dt.float32r
BF16 = mybir.dt.bfloat16
AF = mybir.ActivationFunctionType
OP = mybir.AluOpType

B = 8
CH = 512
H = W = 32
NT = H * W          # 1024 tokens
HEADS = 8
HD = 64
EPS = 1e-6
P = 128
CT = CH // P        # 4 channel tiles
TT = NT // P        # 8 token tiles
IC = NT // 512      # 2 free-dim chunks of 512
NP = HEADS // 2     # 4 head pairs
STRIP_W = 60 * 32   # 1920


def _build_strips(rel: np.ndarray) -> np.ndarray:
    """(3969, 8) rel table -> (8, 128, 1920) bias strips.

    strip[h, 32*jh_l + jw, 32*g + iw] = T_h[g - jh_l + 3, iw - jw + 31]
    where T_h = rel[:, h].reshape(63, 63).
    bias.T block for key-tile jt is then strip[:, (28-4*jt)*32 : +1024].
    """
    T = rel.reshape(63, 63, HEADS)  # [a, b, h]
    jh_l = np.arange(4)[:, None, None, None]
    jw = np.arange(32)[None, :, None, None]
    g = np.arange(60)[None, None, :, None]
    iw = np.arange(32)[None, None, None, :]
    a = g - jh_l + 3          # in [0,62]
    b = iw - jw + 31          # in [0,62]
    a_b, b_b = np.broadcast_arrays(a, b)
    out = T[a_b, b_b, :]      # (4, 32, 60, 32, 8)
    out = np.ascontiguousarray(np.moveaxis(out, -1, 0)).reshape(HEADS, P, STRIP_W)
    return out.astype(np.float32)


def _build_nc() -> bass.Bass:
    nc = bass.Bass()

    x_d = nc.declare_dram_parameter("x", [CH, NT], F32, isOutput=False)
    wqT_d = nc.declare_dram_parameter("wqT", [CH, CH], BF16, isOutput=False)
    wkT_d = nc.declare_dram_parameter("wkT", [CH, CH], BF16, isOutput=False)
    wvT_d = nc.declare_dram_parameter("wvT", [CH, CH], BF16, isOutput=False)
    wpP_d = nc.declare_dram_parameter("wpP", [P, NP, CH], BF16, isOutput=False)
    bqk_d = nc.declare_dram_parameter("bqk", [P, 2, CT], F32, isOutput=False)
    bp_d = nc.declare_dram_parameter("bp", [P, CT], F32, isOutput=False)
    bv_d = nc.declare_dram_parameter("bv", [1, CH], BF16, isOutput=False)
    estr_d = nc.declare_dram_parameter("estr", [HEADS, P, STRIP_W], BF16, isOutput=False)
    y_d = nc.declare_dram_parameter("y", [CH, NT], F32, isOutput=True)

    with tile.TileContext(nc) as tc, ExitStack() as ctx:
        singles = ctx.enter_context(tc.tile_pool(name="singles", bufs=1))
        work = ctx.enter_context(tc.tile_pool(name="work", bufs=2))
        strip_pool = ctx.enter_context(tc.tile_pool(name="strip_pool", bufs=4))
        a_pool = ctx.enter_context(tc.tile_pool(name="a_pool", bufs=4))
        ah1_pool = ctx.enter_context(tc.tile_pool(name="ah1_pool", bufs=6))
        # PSUM: psM 2 slots x (128,1024)f32 = 4 banks; psO 2 x (65,1024) = 4.
        psM = ctx.enter_context(tc.tile_pool(name="psM", bufs=2, space="PSUM"))
        psO = ctx.enter_context(tc.tile_pool(name="psO", bufs=2, space="PSUM"))

        # ---------- persistent SBUF ----------
        x_sb = singles.tile([P, CT, NT], F32)        # raw x (residual + LN apply)
        xb_sb = singles.tile([P, CT, NT], BF16)      # LN output (normalized bf16)
        qT_sb = singles.tile([P, CT, NT], BF16)      # (d part, t free), pair-stacked
        kT_sb = singles.tile([P, CT, NT], BF16)
        v_sb = singles.tile([P, TT, HEADS * (HD + 1)], BF16)  # per head [v(64)|1]
        oTP_sb = singles.tile([P, NP, NT], BF16)     # pair-stacked normalized oT
        wpP_sb = singles.tile([P, NP, CH], BF16)
        bqk_sb = singles.tile([P, 2, CT], F32)
        bp_sb = singles.tile([P, CT], F32)
        bv_sb = singles.tile([1, CH], BF16)
        bvb_sb = singles.tile([P, CH], BF16)         # bv broadcast across partitions
        ones_mat = singles.tile([P, P], BF16)
        ones_row = singles.tile([1, P], BF16)
        eps_sb = singles.tile([P, 1], F32)
        nc.vector.memset(eps_sb[:], float(EPS))
        lnsm_sb = singles.tile([P, 2, 8], F32)       # LN mu/ve small layout
        lnsmb_sb = singles.tile([P, 2, 8], BF16)     # LN rs,b small bf16
        lnrow_sb = singles.tile([1, 2, NT], BF16)    # rs,b rows at partition 0
        lnmv_sb = singles.tile([1, 2, NT], F32)      # mu,ve rows at partition 0
        zrowZ_sb = singles.tile([HD + 1, 2, NT], F32)  # Z rows evac'd at part. 64
        zsm_sb = singles.tile([P, 2, 8], F32)        # Z small layout (per pair)
        zrb_sb = singles.tile([P, 2, 8], BF16)       # 1/Z small, bf16
        zrow_sb = singles.tile([1, 2, NT], BF16)     # 1/Z rows at partition 0

        # x first, in per-ct chunks — the LN cast chases the chunks
        x_r = x_d.rearrange("(ct p) t -> ct p t", p=P)
        for ct in range(CT):
            nc.sync.dma_start(x_sb[:, ct], x_r[ct])
        nc.vector.memset(ones_mat[:], 1.0)
        nc.vector.memset(ones_row[:], 1.0)
        nc.sync.dma_start(bqk_sb[:], bqk_d[:])
        nc.sync.dma_start(bp_sb[:], bp_d[:])
        nc.sync.dma_start(bv_sb[:], bv_d[:])
        nc.sync.dma_start(wpP_sb[:], wpP_d[:])

        # ones columns of v_aug
        v_view = v_sb[:].rearrange("p tt (h w) -> p tt h w", w=HD + 1)
        nc.vector.memset(v_view[:, :, :, HD : HD + 1], 1.0)

        wqT_sb = singles.tile([P, CT, CH], BF16)
        wkT_sb = singles.tile([P, CT, CH], BF16)
        wvT_sb = singles.tile([P, CT, CH], BF16)
        nc.sync.dma_start(wqT_sb[:], wqT_d.rearrange("(ck p) d -> p ck d", p=P))
        nc.sync.dma_start(wkT_sb[:], wkT_d.rearrange("(ck p) d -> p ck d", p=P))
        nc.sync.dma_start(wvT_sb[:], wvT_d.rearrange("(ck p) d -> p ck d", p=P))

        # strips for pairs 0,1 DMA'd in the prologue; pairs 2,3 prefetched
        # from inside the pair loop (keeps the sync queue from stalling on
        # the strip-slot WAR semaphore ahead of the z-chain DMAs)
        estr_tiles = []
        for h in range(HEADS):
            st = strip_pool.tile([P, STRIP_W], BF16, name=f"estr_{h}", tag="strip")
            if h < 4:
                nc.sync.dma_start(st[:], estr_d[h])
            estr_tiles.append(st)

        # PE warmup: dummy matmuls during the x DMA (HAM un-throttle needs
        # ~3.4us of sustained PE activity; these overlap the input DMA).
        warm_ps = psM.tile([P, NT], F32, tag="s")
        for i in range(16):
            nc.tensor.matmul(warm_ps[:, :P], lhsT=ones_mat[:], rhs=ones_mat[:],
                             start=True, stop=True, skip_group_check=True)
        # bv broadcast across partitions (K=1 ones-column matmul + ACT evac)
        nc.tensor.matmul(warm_ps[:, :CH], lhsT=ones_row[:], rhs=bv_sb[:],
                         start=True, stop=True, skip_group_check=True)
        nc.scalar.activation(out=bvb_sb[:], in_=warm_ps[:, :CH], func=AF.Copy)

        # ---------- phase 1: LayerNorm ----------
        # bf16 cast of raw x (DVE, pipelined with the x DMA) + squares on
        # ACT (idle in this phase); stats via bf16 ones-matmuls
        with tc.tile_pool(name="ln_pool", bufs=2) as lnp, \
             tc.tile_pool(name="ln_single", bufs=1) as lns:
            # xb_sb temporarily holds the raw-x bf16 cast (overwritten by the
            # normalized output after the stats matmuls complete); cast,
            # square, and stats matmuls interleaved per channel tile
            sum_ps = psM.tile([P, NT], F32, tag="s")
            sq_ps = psM.tile([P, NT], F32, tag="s")
            for ct in range(CT):
                nc.vector.tensor_copy(out=xb_sb[:, ct], in_=x_sb[:, ct])
                x2 = lnp.tile([P, NT], BF16, name=f"x2_{ct}", tag="x2", bufs=2)
                nc.scalar.activation(out=x2[:], in_=xb_sb[:, ct], func=AF.Square)
                for ic in range(IC):
                    sl = slice(ic * 512, ic * 512 + 512)
                    nc.tensor.matmul(sum_ps[:, sl], lhsT=ones_mat[:],
                                     rhs=xb_sb[:, ct, sl],
                                     start=(ct == 0), stop=(ct == CT - 1))
                    nc.tensor.matmul(sq_ps[:, sl], lhsT=ones_mat[:],
                                     rhs=x2[:, sl],
                                     start=(ct == 0), stop=(ct == CT - 1))

            # LN scalars on a DMA-redistributed (128, 8) small layout: the
            # replicated rows of sum/sq go through ACT copies, one row is
            # DMA'd small, rs = 1/sqrt(var+eps) and b = -mu*rs cost ~100ns
            # each there (DVE recip is ~6 cyc per FREE elem), then rows are
            # DMA'd back and partition-broadcast by a stride-0 DMA.
            rs_bc = lns.tile([P, NT], F32)
            b_bc = lns.tile([P, NT], F32)
            nc.scalar.activation(out=lnmv_sb[:, 0, :], in_=sum_ps[0:1, :],
                                 func=AF.Copy, scale=1.0 / CH)
            nc.scalar.activation(out=lnmv_sb[:, 1, :], in_=sq_ps[0:1, :],
                                 func=AF.Copy, scale=1.0 / CH)
            for e in range(2):
                nc.scalar.dma_start(
                    lnsm_sb[:, e, :],
                    lnmv_sb[:, e, :].rearrange("o (p j) -> o p j", j=8))
            musq = lns.tile([P, 8], F32)
            var_s = lns.tile([P, 8], F32)
            rs_s = lns.tile([P, 8], F32)
            b_s = lns.tile([P, 8], F32)
            nc.vector.tensor_tensor(out=musq[:], in0=lnsm_sb[:, 0, :],
                                    in1=lnsm_sb[:, 0, :], op=OP.mult)
            nc.vector.tensor_tensor(out=var_s[:], in0=lnsm_sb[:, 1, :],
                                    in1=musq[:], op=OP.subtract)
            nc.scalar.activation(out=var_s[:], in_=var_s[:], func=AF.Sqrt,
                                 bias=eps_sb[:])
            nc.vector.reciprocal(out=rs_s[:], in_=var_s[:])
            nc.vector.scalar_tensor_tensor(out=b_s[:], in0=lnsm_sb[:, 0, :],
                                           scalar=-1.0, in1=rs_s[:],
                                           op0=OP.mult, op1=OP.mult)
            nc.vector.tensor_copy(out=lnsmb_sb[:, 0, :], in_=rs_s[:])
            nc.vector.tensor_copy(out=lnsmb_sb[:, 1, :], in_=b_s[:])
            for e in range(2):
                nc.gpsimd.dma_start(
                    lnrow_sb[:, e, :].rearrange("o (p j) -> o p j", j=8),
                    lnsmb_sb[:, e, :])
            # HAM filler: keep the PE array active across the LN small-DMA
            # chain so the V/QK phases start at full clock
            warm2 = psM.tile([P, NT], F32, name="warm2", tag="s")
            for i in range(28):
                nc.tensor.matmul(warm2[:, :P], lhsT=ones_mat[:], rhs=ones_mat[:],
                                 start=True, stop=True, skip_group_check=True)
            # replicate rs,b across partitions: K=1 matmuls + ACT evacuation
            rep_ps = psM.tile([P, NT], F32, name="lnrep_ps", tag="s")
            bep_ps = psM.tile([P, NT], F32, name="lnbep_ps", tag="s")
            for ic in range(IC):
                sl = slice(ic * 512, ic * 512 + 512)
                nc.tensor.matmul(rep_ps[:, sl], lhsT=ones_mat[0:1, :],
                                 rhs=lnrow_sb[:, 0, sl], start=True, stop=True)
                nc.tensor.matmul(bep_ps[:, sl], lhsT=ones_mat[0:1, :],
                                 rhs=lnrow_sb[:, 1, sl], start=True, stop=True)
            nc.scalar.activation(out=rs_bc[:], in_=rep_ps[:], func=AF.Copy)
            nc.scalar.activation(out=b_bc[:], in_=bep_ps[:], func=AF.Copy)

            # apply xb = x*rs + b, split across DVE and GPSIMD (crosswise so
            # each ct's chain spans both engines and they run in parallel)
            for ct in range(CT):
                xm = lnp.tile([P, NT], F32, name=f"xm_{ct}", tag="xm", bufs=2)
                e_mul = nc.vector if ct < 2 else nc.gpsimd
                e_add = nc.gpsimd if ct < 2 else nc.vector
                e_mul.tensor_tensor(out=xm[:], in0=x_sb[:, ct], in1=rs_bc[:],
                                    op=OP.mult)
                e_add.tensor_tensor(out=xb_sb[:, ct], in0=xm[:], in1=b_bc[:],
                                    op=OP.add)

        # ---------- phase 2a: V projection ----------
        # token tiles 0,1 up front; 2..7 interleaved into pair 0's jt loop
        # (two iterations ahead of their first consumer) to absorb the
        # otherwise-serial V phase into the attention pipeline
        def emit_v(tt):
            tsl = slice(tt * P, tt * P + P)
            v_ps = psM.tile([P, NT], F32, name=f"v_ps_{tt}", tag="s")
            for ck in range(CT):
                nc.tensor.matmul(v_ps[:, :512], lhsT=xb_sb[:, ck, tsl],
                                 rhs=wvT_sb[:, ck, :],
                                 start=(ck == 0), stop=(ck == CT - 1))
            vp_v = v_ps[:, :512].rearrange("p (h w) -> p h w", w=HD)
            bv_v = bvb_sb[:].rearrange("p (h w) -> p h w", w=HD)
            nc.vector.scalar_tensor_tensor(
                out=v_view[:, tt, :, :HD], in0=vp_v, scalar=0.0,
                in1=bv_v, op0=OP.bypass, op1=OP.add)

        emit_v(0)
        emit_v(1)

        # ---------- phases 2b+3: per head pair: Q/K proj then attention ----------
        def emit_znorm(g, ob0, ob1):
            """Replicate 1/Z rows (K=1 matmuls) and normalize into oTP.

            Emitted one pair LATE (mid next pair's jt loop) so the z-chain's
            DMA round-trip latency is hidden behind PE work instead of
            stalling the in-order PE queue.
            """
            zr0 = psM.tile([P, NT], F32, name=f"zr0_{g}", tag="s")
            zr1 = psM.tile([P, NT], F32, name=f"zr1_{g}", tag="s")
            for ic in range(IC):
                sl = slice(ic * 512, ic * 512 + 512)
                nc.tensor.matmul(zr0[:HD, sl], lhsT=ones_mat[0:1, :HD],
                                 rhs=zrow_sb[:, 0, sl], start=True, stop=True)
                nc.tensor.matmul(zr1[:HD, sl], lhsT=ones_mat[0:1, :HD],
                                 rhs=zrow_sb[:, 1, sl], start=True, stop=True)
            nc.vector.tensor_tensor(out=oTP_sb[:HD, g], in0=ob0[:],
                                    in1=zr0[:HD, :], op=OP.mult)
            tmpO = work.tile([HD, NT], BF16, name=f"tmpO_{g}", tag="tmpO")
            nc.vector.tensor_tensor(out=tmpO[:], in0=ob1[:],
                                    in1=zr1[:HD, :], op=OP.mult)
            nc.gpsimd.dma_start(oTP_sb[HD:, g], tmpO[:])

        def emit_qk(gq, which, ic):
            """One Q or K projection chunk (4-MM accumulation + evac)."""
            sl = slice(ic * 512, ic * 512 + 512)
            dq = slice(gq * P, gq * P + P)
            w_sb = wqT_sb if which == 0 else wkT_sb
            dst = qT_sb if which == 0 else kT_sb
            ps = psM.tile([P, NT], F32, name=f"qk_ps_{gq}_{which}_{ic}", tag="s")
            for ck in range(CT):
                nc.tensor.matmul(ps[:, :512], lhsT=w_sb[:, ck, dq],
                                 rhs=xb_sb[:, ck, sl],
                                 start=(ck == 0), stop=(ck == CT - 1))
            nc.vector.tensor_scalar_add(out=dst[:, gq, sl], in0=ps[:, :512],
                                        scalar1=bqk_sb[:, which, gq : gq + 1])

        # Q/K for pair 0 up front
        for ic in range(IC):
            emit_qk(0, 0, ic)
            emit_qk(0, 1, ic)

        pend = None
        for g in range(NP):
            h0, h1 = 2 * g, 2 * g + 1
            # prefetch strips for pair g+2
            if g < 2:
                nc.sync.dma_start(estr_tiles[2 * g + 4][:], estr_d[2 * g + 4])
                nc.sync.dma_start(estr_tiles[2 * g + 5][:], estr_d[2 * g + 5])
            # Q/K chunks for pair g+1, interleaved into this pair's jt loop
            # (fills PE stall slivers so the HAM clock gate stays warm)
            qk_fill = ([(g + 1, w, ic) for ic in range(IC) for w in (0, 1)]
                       if g < NP - 1 else [])

            # attention for heads (h0: partitions 0-63, h1: 64-127)
            o0 = psO.tile([HD + 1, NT], F32, name=f"o0_{g}", tag="o")
            o1 = psO.tile([HD + 1, NT], F32, name=f"o1_{g}", tag="o")
            ah0 = {}
            ah1 = {}
            for jt in range(TT):
                jsl = slice(jt * P, jt * P + P)
                off = (28 - 4 * jt) * 32
                if g == 0 and jt < TT - 2:
                    emit_v(jt + 2)
                # pending z-normalization of the PREVIOUS pair (its DMA
                # chain has had ~3 jt iterations of PE work to complete)
                if jt == 3 and pend is not None:
                    emit_znorm(*pend)
                    pend = None
                # paired score matmuls: (0,0) and (64,0) row tiles run
                # concurrently on the PE array
                s0 = psM.tile([P, NT], F32, name=f"s0_{g}_{jt}", tag="s")
                s1 = psM.tile([P, NT], F32, name=f"s1_{g}_{jt}", tag="s")
                # grouped per head (A0,A1 then B0,B1): each head's lhsT loads
                # once and head B's LDWEIGHTS pulls ahead during A's MMs
                # (different row groups); a fully alternating order measured
                # ~35us WORSE (per-MM weight reloads, no pull-ahead)
                for ic in range(IC):
                    sl = slice(ic * 512, ic * 512 + 512)
                    nc.tensor.matmul(s0[:, sl], lhsT=kT_sb[:HD, g, jsl],
                                     rhs=qT_sb[:HD, g, sl], start=True, stop=True)
                for ic in range(IC):
                    sl = slice(ic * 512, ic * 512 + 512)
                    nc.tensor.matmul(s1[:, sl], lhsT=kT_sb[HD:, g, jsl],
                                     rhs=qT_sb[HD:, g, sl], start=True, stop=True)
                ah1[jt] = ah1_pool.tile([P, NT], BF16, name=f"ah1_{g}_{jt}",
                                        tag="ah1")
                aT0 = a_pool.tile([P, NT], BF16, name=f"aT0_{g}_{jt}", tag="aT")
                aT1 = a_pool.tile([P, NT], BF16, name=f"aT1_{g}_{jt}", tag="aT")
                nc.scalar.activation(out=aT0[:], in_=s0[:], func=AF.Exp)
                nc.scalar.activation(out=aT1[:], in_=s1[:], func=AF.Exp)
                ah0[jt] = a_pool.tile([P, NT], BF16, name=f"ab0_{g}_{jt}",
                                      tag="ab", bufs=6)
                nc.vector.tensor_tensor(out=ah0[jt][:], in0=aT0[:],
                                        in1=estr_tiles[h0][:, off : off + NT],
                                        op=OP.mult)
                nc.vector.tensor_tensor(out=ah1[jt][:], in0=aT1[:],
                                        in1=estr_tiles[h1][:, off : off + NT],
                                        op=OP.mult)
                # attn@V for h0, also two jt iterations behind on the
                # stored aT tile so these MMs never wait on DVE either
                if jt >= 2:
                    for ic in range(IC):
                        sl = slice(ic * 512, ic * 512 + 512)
                        nc.tensor.matmul(
                            o0[:, sl],
                            lhsT=v_sb[:, jt - 2,
                                      h0 * (HD + 1) : (h0 + 1) * (HD + 1)],
                            rhs=ah0[jt - 2][:, sl],
                            start=(jt == 2), stop=False)
                # attn@V for h1, two jt iterations behind (its aT tiles
                # are stored, so these MMs never wait on DVE)
                if jt >= 2:
                    for ic in range(IC):
                        sl = slice(ic * 512, ic * 512 + 512)
                        nc.tensor.matmul(
                            o1[:, sl],
                            lhsT=v_sb[:, jt - 2,
                                      h1 * (HD + 1) : (h1 + 1) * (HD + 1)],
                            rhs=ah1[jt - 2][:, sl],
                            start=(jt == 2), stop=False)
                # PE gap-filler: one Q/K chunk of the next pair per odd jt
                if jt % 2 == 1 and qk_fill:
                    emit_qk(*qk_fill.pop(0))

            # drain the lagged accumulations (jt 6, 7, both heads)
            for jd in (TT - 2, TT - 1):
                for ic in range(IC):
                    sl = slice(ic * 512, ic * 512 + 512)
                    nc.tensor.matmul(
                        o0[:, sl],
                        lhsT=v_sb[:, jd, h0 * (HD + 1) : (h0 + 1) * (HD + 1)],
                        rhs=ah0[jd][:, sl],
                        start=False, stop=(jd == TT - 1))
            for jd in (TT - 2, TT - 1):
                for ic in range(IC):
                    sl = slice(ic * 512, ic * 512 + 512)
                    nc.tensor.matmul(
                        o1[:, sl],
                        lhsT=v_sb[:, jd, h1 * (HD + 1) : (h1 + 1) * (HD + 1)],
                        rhs=ah1[jd][:, sl],
                        start=False, stop=(jd == TT - 1))

            # evacuate h0: unnormalized oT (bf16) + Z row
            ob0 = work.tile([HD, NT], BF16, name=f"ob0_{g}", tag="ob")
            nc.scalar.activation(out=ob0[:], in_=o0[:HD, :], func=AF.Copy)
            nc.vector.tensor_copy(out=zrowZ_sb[HD : HD + 1, 0, :],
                                  in_=o0[HD : HD + 1, :])
            zdma = nc.sync if g == NP - 1 else nc.gpsimd
            zdma.dma_start(
                zsm_sb[:, 0, :],
                zrowZ_sb[HD : HD + 1, 0, :].rearrange("o (p j) -> o p j", j=8))

            # ---- z-chain: 1/Z rows via DMA-redistribution ----
            # DMA Z into a (128, 16) layout (DVE recip is ~6 cycles per FREE
            # element, so keep the free dim tiny), recip, cast bf16, DMA
            # back to partition-0 rows for the K=1 replicate matmuls
            ob1 = work.tile([HD, NT], BF16, name=f"ob1_{g}", tag="ob")
            nc.scalar.activation(out=ob1[:], in_=o1[:HD, :], func=AF.Copy)
            nc.vector.tensor_copy(out=zrowZ_sb[HD : HD + 1, 1, :],
                                  in_=o1[HD : HD + 1, :])
            zdma.dma_start(
                zsm_sb[:, 1, :],
                zrowZ_sb[HD : HD + 1, 1, :].rearrange("o (p j) -> o p j", j=8))
            nc.vector.reciprocal(out=zsm_sb[:], in_=zsm_sb[:])
            nc.vector.tensor_copy(out=zrb_sb[:], in_=zsm_sb[:])
            for e in range(2):
                zdma.dma_start(
                    zrow_sb[:, e, :].rearrange("o (p j) -> o p j", j=8),
                    zrb_sb[:, e, :])
            pend = (g, ob0, ob1)

        # HAM filler across pair-3's z-chain so proj runs at full clock
        warm3 = psM.tile([P, NT], F32, name="warm3", tag="s")
        for i in range(12):
            nc.tensor.matmul(warm3[:, :P], lhsT=ones_mat[:], rhs=ones_mat[:],
                             start=True, stop=True, skip_group_check=True)
        emit_znorm(*pend)

        # ---------- phase 4: output projection + residual ----------
        for ct in range(CT):
            csl = slice(ct * P, ct * P + P)
            for ic in range(IC):
                sl = slice(ic * 512, ic * 512 + 512)
                y_ps = psM.tile([P, NT // 2], F32, name=f"y_ps_{ct}_{ic}",
                                tag="s")
                for g in range(NP):
                    nc.tensor.matmul(y_ps[:], lhsT=wpP_sb[:, g, csl],
                                     rhs=oTP_sb[:, g, sl],
                                     start=(g == 0), stop=(g == NP - 1))
                y_sb = work.tile([P, 512], F32, name=f"y_{ct}_{ic}", tag="y")
                nc.vector.scalar_tensor_tensor(
                    out=y_sb[:], in0=y_ps[:], scalar=bp_sb[:, ct : ct + 1],
                    in1=x_sb[:, ct, sl], op0=OP.add, op1=OP.add)
                nc.sync.dma_start(y_d[csl, sl], y_sb[:])

    return nc


def _legalize_waits(nc, max_waits: int = 1):
    """Split multi-wait instructions into preceding same-engine NoOps.

    The TPB instruction encoding carries a single sync-wait slot and this
    walrus build refuses to legalize ("Too many sync wait commands"), so do
    it here: engines execute their queue in order, so a NoOp carrying one of
    the waits delays everything after it on that engine identically.
    """
    import orjson

    data = orjson.loads(mybir.module_to_json_bytes(nc.m))
    ctr = [0]

    def fix_block(block):
        out = []
        for inst in block.get("instructions", []):
            si = inst.get("sync_info") or {}
            waits = si.get("on_wait") or []
            if len(waits) > max_waits:
                for w in waits[max_waits:]:
                    ctr[0] += 1
                    nop = {
                        "name": f"I-WS{ctr[0]}",
                        "opcode": "NoOp",
                        "engine": inst["engine"],
                        "ins": [],
                        "outs": [],
                        "sync_info": {"on_wait": [w], "on_update": []},
                    }
                    if "debug" in inst:
                        nop["debug"] = inst["debug"]
                    out.append(nop)
                si = dict(si)
                si["on_wait"] = waits[:max_waits]
                inst["sync_info"] = si
            out.append(inst)
        block["instructions"] = out
        for b in block.get("blocks", []):
            fix_block(b)

    for fn in data["functions"]:
        for b in fn.get("blocks", []):
            fix_block(b)
    nc.m = mybir.module_from_json_bytes(orjson.dumps(data))
    return nc


_NC = None


def _host_prep(x, norm_w, norm_b, wq, bq, wk, bk, wv, bv, wp, bp, rel):
    scale = HD ** -0.5
    BF = ml_dtypes.bfloat16
    # fold LN affine + score scale into the projection weights (exact algebra)
    wq_eff = (wq * norm_w[None, :]) * scale
    bq_eff = (bq + wq @ norm_b) * scale
    wk_eff = wk * norm_w[None, :]
    bk_eff = bk + wk @ norm_b
    wv_eff = wv * norm_w[None, :]
    bv_eff = bv + wv @ norm_b

    wqT = np.ascontiguousarray(wq_eff.T).astype(BF)
    wkT = np.ascontiguousarray(wk_eff.T).astype(BF)
    wvT = np.ascontiguousarray(wv_eff.T).astype(BF)
    # wp pair-stacked: wpP[p, g, c] = wp.T[(2g + p//64)*64 + p%64, c]
    wpP = np.ascontiguousarray(
        wp.T.reshape(NP, P, CH)
    ).transpose(1, 0, 2)
    wpP = np.ascontiguousarray(wpP).astype(BF)

    bqk = np.stack([bq_eff.reshape(CT, P).T, bk_eff.reshape(CT, P).T],
                   axis=1).astype(np.float32)          # (P, 2, CT)
    bp_a = np.ascontiguousarray(bp.reshape(CT, P).T).astype(np.float32)  # (P, CT)
    bv_a = bv_eff.reshape(1, CH).astype(BF)
    estr = np.exp(_build_strips(np.asarray(rel, np.float32))).astype(BF)

    shared = {
        "wqT": wqT, "wkT": wkT, "wvT": wvT, "wpP": wpP,
        "bqk": bqk, "bp": bp_a, "bv": bv_a, "estr": estr,
    }
    in_maps = []
    for b in range(B):
        m = dict(shared)
        m["x"] = np.ascontiguousarray(x[b].reshape(CH, NT)).astype(np.float32)
        in_maps.append(m)
    return in_maps


def kernel(**inputs):
    global _NC
    if _NC is None:
        _NC = _legalize_waits(_build_nc())
    in_maps = _host_prep(**{k: np.asarray(v) for k, v in inputs.items()})
    res = run_bass_kernel_spmd(_NC, in_maps, list(range(B)))
    out = np.stack([res.results[b]["y"].reshape(CH, H, W) for b in range(B)])
    return out.astype(np.float32)


if __name__ == "__main__":
    nc = _build_nc()
    print("built OK")
